# revision 12
# baseline (speedup 1.0000x reference)
"""Trainium2 Bass kernel for BeliefPropagationCV (LDPC check-node update).

Math: out[b,o] = 2*atanh(clip(prod_i (mask[o,i]*x[b,i] + 1-mask[o,i])))

The product over masked entries is computed in log-domain so it becomes two
matmuls over the Tanner graph mask:
    lnsq[b,i] = ln(x[b,i]^2) = 2*ln|x[b,i]|
    L2[b,o]   = sum_i mask[o,i]*lnsq[b,i]      (matmul)
    N[b,o]    = sum_i mask[o,i]*(x[b,i] < 0)   (matmul, negative-factor count)
    prod      = (-1)^N * exp(L2/2)
    out       = sign * (ln(1+t) - ln(1-t)),  t = min(exp(L2/2), 1-1e-7)

lnsq is split hi/lo into two bf16 matrices (hi = bf16(lnsq), lo = bf16(lnsq-hi))
so the matmuls run at full bf16 PE rate while retaining ~fp32 accuracy (the
mask is 0/1, exact in bf16; accumulation is fp32 in PSUM).

Sharding: output-dim (check-node rows of the mask) across 8 cores. Each core
gets the full x [128,2048] plus a [128,2048] row-shard of the mask, and
produces out.T shard [128(o),128(b)]. Host concatenates and transposes.
This minimizes HBM traffic (2MB/core) vs batch sharding (8.1MB/core).

Scheduling notes (walrus codegen allows ONE semaphore wait per engine
instruction): PSUM reads serialize cross-engine and pool-slot recycling waits
on all previous readers, so PSUM pools are sized to never recycle a slot
whose readers span two engines, and op emission order is chosen so each
instruction needs at most one new semaphore tick.
"""

import os
import sys
from contextlib import ExitStack

import numpy as np

for _p in ("/opt/trn_rl_repo", "/root/.axon_site/_ro/trn_rl_repo"):
    if os.path.isdir(_p) and _p not in sys.path:
        sys.path.append(_p)

import concourse.bacc as bacc
import concourse.bass as bass
import concourse.tile as tile
from concourse import mybir
from concourse.bass_utils import run_bass_kernel_spmd
from concourse.masks import make_identity
from concourse.tile_rust import add_dep_helper


class StreamOrder:
    """Pins per-engine instruction order with nosync edges so the scheduler
    keeps emission order; semaphore waits then coalesce to <=1 per
    instruction (the walrus codegen limit)."""

    def __init__(self):
        self.last: dict = {}

    def add(self, key, binst):
        ins = getattr(binst, "ins", binst)
        prev = self.last.get(key)
        if prev is not None:
            add_dep_helper(ins, prev, sync=False, reason="stream-order")
        self.last[key] = ins
        return binst

N_CORES = 8
B = 128          # batch
O = 1024         # check nodes (mask rows)
I = 2048         # variable-node messages (mask cols)
OS = O // N_CORES  # mask rows per core

F32 = mybir.dt.float32
BF16 = mybir.dt.bfloat16
AF = mybir.ActivationFunctionType
ALU = mybir.AluOpType
CLIP = float(np.float32(1.0) - np.float32(1e-7))

N_GROUPS = 2
GW = I // N_GROUPS   # 1024 columns per super-group
GC = GW // 128       # 8 chunks per super-group
N_CHUNKS = I // 128  # 16 k-chunks of 128


def build_body(ctx: ExitStack, tc: "tile.TileContext", o_d, x_d, m_d):
    """Emit the per-core program. o_d: [OS,B] f32 out; x_d: [B,I]; m_d: [OS,I]."""
    nc = tc.nc
    ts = bass.ts

    const = ctx.enter_context(tc.tile_pool(name="const", bufs=1))
    big = ctx.enter_context(tc.tile_pool(name="big", bufs=1))
    smal = ctx.enter_context(tc.tile_pool(name="smal", bufs=1))
    # PSUM dep-tracking is per-TILE: one tile per super-group per operand,
    # never recycled. Budget (8 banks): px 2x[128,1024]f32=4, pm
    # 2x[128,1024]bf16=2, po=1, warmup=1.
    psx = ctx.enter_context(tc.tile_pool(name="psx", bufs=N_GROUPS, space="PSUM"))
    psm = ctx.enter_context(tc.tile_pool(name="psm", bufs=N_GROUPS, space="PSUM"))
    pso = ctx.enter_context(tc.tile_pool(name="pso", bufs=1, space="PSUM"))
    pswarm = ctx.enter_context(tc.tile_pool(name="pswarm", bufs=1, space="PSUM"))

    so = StreamOrder()
    pe, act, dve = "PE", "ACT", "DVE"

    # bf16 identity first, f32 second: the warmup transpose (f32 ident, the
    # last GPSIMD product) lets the PE observe the whole GPSIMD tick range, so
    # later transposes only carry their DMA wait.
    identb = const.tile([128, 128], BF16)
    make_identity(nc, identb[:])
    ident = const.tile([128, 128], F32)
    make_identity(nc, ident[:])
    pw = pswarm.tile([128, 128], F32)
    so.add(pe, nc.tensor.transpose(pw[:], ident[:], ident[:]))

    x_sb = big.tile([128, I], F32, tag="x")
    nc.sync.dma_start(x_sb[:], x_d[:])
    m_bf = big.tile([128, I], BF16, tag="m")
    nc.gpsimd.dma_start(m_bf[:], m_d[:])  # SWDGE casts f32->bf16 in flight

    sq = big.tile([128, I], F32, tag="sq")        # xT^2
    lnsq = big.tile([128, I], F32, tag="lnsq")    # 2*ln|x|, transposed layout
    maskT = big.tile([128, I], BF16, tag="maskT")  # mask shard transposed
    rhs = big.tile([128, N_CHUNKS * 384], BF16, tag="rhs")  # [hi|lo|neg]/chunk
    rhs3 = rhs[:].rearrange("p (c n) -> p c n", n=384)

    for g in range(N_GROUPS):
        gsl = slice(g * GW, (g + 1) * GW)
        cs = slice(GC * g, GC * g + GC)
        px = psx.tile([128, GW], F32, tag="px")
        for j in range(GC):
            so.add(pe, nc.tensor.transpose(px[:, ts(j, 128)], x_sb[:, ts(GC * g + j, 128)], ident[:]))
        pm = psm.tile([128, GW], BF16, tag="pm")
        for j in range(GC):
            so.add(pe, nc.tensor.transpose(pm[:, ts(j, 128)], m_bf[:, ts(GC * g + j, 128)], identb[:]))
        # DVE observes the PE tick here (covers px too); sole pm reader.
        so.add(dve, nc.vector.tensor_copy(maskT[:, gsl], pm[:]))
        # ACT: first px reader.
        so.add(act, nc.scalar.activation(sq[:, gsl], px[:], AF.Square))
        so.add(act, nc.scalar.activation(lnsq[:, gsl], sq[:, gsl], AF.Ln))
        # DVE: second px reader (ACT serialization wait; PE tick observed).
        pxv = px[:].rearrange("p (c n) -> p c n", n=128)
        so.add(dve, nc.vector.tensor_scalar(rhs3[:, cs, 256:384], pxv, 0.0, None, ALU.is_lt))
        lv = lnsq[:, gsl].rearrange("p (c n) -> p c n", n=128)
        so.add(dve, nc.vector.tensor_copy(rhs3[:, cs, 0:128], lv))  # hi (bf16 round)
        so.add(dve, nc.vector.tensor_tensor(rhs3[:, cs, 128:256], lv, rhs3[:, cs, 0:128], ALU.subtract))  # lo

    po = pso.tile([128, 384], F32, tag="po")
    for c in range(N_CHUNKS):
        so.add(pe, nc.tensor.matmul(
            po[:], maskT[:, ts(c, 128)], rhs3[:, c, :],
            start=(c == 0), stop=(c == N_CHUNKS - 1),
        ))

    # epilogue: [128(o), 128(b)] tiles. All PSUM reads on DVE (serialize free);
    # only one tensor op input may come from PSUM -> stage lo-sums via SBUF.
    lo_sb = smal.tile([128, B], F32, tag="lo_sb")
    so.add(dve, nc.vector.tensor_copy(lo_sb[:], po[:, 128:256]))
    l2 = smal.tile([128, B], F32, tag="l2")
    so.add(dve, nc.vector.tensor_add(l2[:], po[:, 0:128], lo_sb[:]))
    pari = smal.tile([128, B], mybir.dt.int32, tag="pari")
    so.add(dve, nc.vector.tensor_copy(pari[:], po[:, 256:384]))  # exact int count
    par = smal.tile([128, B], mybir.dt.int32, tag="par")
    so.add(dve, nc.vector.tensor_scalar(par[:], pari[:], 1, None, ALU.bitwise_and))
    t = smal.tile([128, B], F32, tag="t")
    so.add(act, nc.scalar.activation(t[:], l2[:], AF.Exp, scale=0.5))
    t2 = smal.tile([128, B], F32, tag="t2")
    so.add(dve, nc.vector.tensor_scalar_min(t2[:], t[:], CLIP))
    a = smal.tile([128, B], F32, tag="a")
    so.add(act, nc.scalar.activation(a[:], t2[:], AF.Ln, bias=1.0))
    bb = smal.tile([128, B], F32, tag="bb")
    so.add(act, nc.scalar.activation(bb[:], t2[:], AF.Ln, bias=1.0, scale=-1.0))
    u = smal.tile([128, B], F32, tag="u")
    so.add(dve, nc.vector.tensor_sub(u[:], a[:], bb[:]))
    sgn = smal.tile([128, B], F32, tag="sgn")
    so.add(dve, nc.vector.tensor_scalar(sgn[:], par[:], -2.0, 1.0, ALU.mult, ALU.add))
    ot = smal.tile([128, B], F32, tag="ot")
    so.add(dve, nc.vector.tensor_mul(ot[:], u[:], sgn[:]))
    nc.sync.dma_start(o_d[:], ot[:])


def build(loop_n: int = 0) -> bass.Bass:
    """Build the SPMD program. loop_n>0 wraps the body in a HW loop (timing)."""
    nc = bacc.Bacc("TRN2", target_bir_lowering=False, debug=False,
                   num_devices=N_CORES)
    x_d = nc.dram_tensor("x", [B, I], F32, kind="ExternalInput").ap()
    m_d = nc.dram_tensor("mask", [OS, I], F32, kind="ExternalInput").ap()
    o_d = nc.dram_tensor("outT", [OS, B], F32, kind="ExternalOutput").ap()
    with tile.TileContext(nc) as tc:
        with ExitStack() as ctx:
            if loop_n > 0:
                with tc.For_i(0, loop_n, 1):
                    build_body(ctx, tc, o_d, x_d, m_d)
            else:
                build_body(ctx, tc, o_d, x_d, m_d)
    nc.compile()
    return nc


_CACHE: dict = {}


def kernel(x: np.ndarray, mask: np.ndarray) -> np.ndarray:
    nc = _CACHE.get("nc")
    if nc is None:
        nc = _CACHE["nc"] = build()
    x = np.ascontiguousarray(np.asarray(x), dtype=np.float32)
    mask = np.ascontiguousarray(np.asarray(mask), dtype=np.float32)
    in_maps = [
        {"x": x, "mask": mask[c * OS:(c + 1) * OS]} for c in range(N_CORES)
    ]
    res = run_bass_kernel_spmd(nc, in_maps, list(range(N_CORES)))
    outT = np.concatenate(
        [res.results[c]["outT"] for c in range(N_CORES)], axis=0
    )  # [O, B]
    return np.ascontiguousarray(outT.T)


# revision 24
# speedup vs baseline: 1.0806x; 1.0806x over previous
"""Trainium2 Bass kernel for BeliefPropagationCV (LDPC check-node update).

Math: out[b,o] = 2*atanh(clip(prod_i (mask[o,i]*x[b,i] + 1-mask[o,i])))

The product over masked entries is computed in log-domain so it becomes two
matmuls over the Tanner graph mask:
    lnsq[b,i] = ln(x[b,i]^2) = 2*ln|x[b,i]|
    L2[b,o]   = sum_i mask[o,i]*lnsq[b,i]      (matmul)
    N[b,o]    = sum_i mask[o,i]*(x[b,i] < 0)   (matmul, negative-factor count)
    prod      = (-1)^N * exp(L2/2)
    out       = sign * (ln(1+t) - ln(1-t)),  t = min(exp(L2/2), 1-1e-7)

lnsq is split hi/lo into two bf16 matrices (hi = bf16(lnsq), lo = bf16(lnsq-hi))
so the matmuls run at full bf16 PE rate while retaining ~fp32 accuracy (the
mask is 0/1, exact in bf16; accumulation is fp32 in PSUM).

Sharding: output-dim (check-node rows of the mask) across 8 cores. Each core
gets the full x [128,2048] plus a [128,2048] row-shard of the mask, and
produces out.T shard [128(o),128(b)]. Host concatenates and transposes.
This minimizes HBM traffic (2MB/core) vs batch sharding (8.1MB/core).

Scheduling notes (walrus codegen allows ONE semaphore wait per engine
instruction): PSUM reads serialize cross-engine and pool-slot recycling waits
on all previous readers, so PSUM pools are sized to never recycle a slot
whose readers span two engines, and op emission order is chosen so each
instruction needs at most one new semaphore tick.
"""

import os
import sys
from contextlib import ExitStack

import numpy as np

for _p in ("/opt/trn_rl_repo", "/root/.axon_site/_ro/trn_rl_repo"):
    if os.path.isdir(_p) and _p not in sys.path:
        sys.path.append(_p)

import concourse.bacc as bacc
import concourse.bass as bass
import concourse.tile as tile
from concourse import mybir
from concourse.bass_utils import run_bass_kernel_spmd
from concourse.masks import make_identity
from concourse.hw_specs import get_activation_tables
from concourse.tile_rust import add_dep_helper


class StreamOrder:
    """Pins per-engine instruction order with nosync edges so the scheduler
    keeps emission order; semaphore waits then coalesce to <=1 per
    instruction (the walrus codegen limit)."""

    def __init__(self):
        self.last: dict = {}

    def add(self, key, binst):
        ins = getattr(binst, "ins", binst)
        prev = self.last.get(key)
        if prev is not None:
            add_dep_helper(ins, prev, sync=False, reason="stream-order")
        self.last[key] = ins
        return binst

N_CORES = 8
B = 128          # batch
O = 1024         # check nodes (mask rows)
I = 2048         # variable-node messages (mask cols)
OS = O // N_CORES  # mask rows per core

F32 = mybir.dt.float32
BF16 = mybir.dt.bfloat16
FP16 = mybir.dt.float16
AF = mybir.ActivationFunctionType
ALU = mybir.AluOpType
CLIP = float(np.float32(1.0) - np.float32(1e-7))

N_GROUPS = 4
GW = I // N_GROUPS   # 512 columns per x-group
GC = GW // 128       # 4 chunks per x-group
N_CHUNKS = I // 128  # 16 k-chunks of 128


def build_body(ctx: ExitStack, tc: "tile.TileContext", o_d, x_d, m_d):
    """Emit the per-core program. o_d: [OS,B] f32 out; x_d: [B,I]; m_d: [OS,I]."""
    nc = tc.nc
    ts = bass.ts

    const = ctx.enter_context(tc.tile_pool(name="const", bufs=1))
    big = ctx.enter_context(tc.tile_pool(name="big", bufs=1))
    smal = ctx.enter_context(tc.tile_pool(name="smal", bufs=1))
    # PSUM dep-tracking is per-TILE: one tile per group per operand, never
    # recycled. Budget (8 banks): px 4x[128,512]f32=4, pm [128,2048]fp16=2,
    # po=1, warmup=1.
    psx = ctx.enter_context(tc.tile_pool(name="psx", bufs=N_GROUPS, space="PSUM"))
    psm = ctx.enter_context(tc.tile_pool(name="psm", bufs=1, space="PSUM"))
    pso = ctx.enter_context(tc.tile_pool(name="pso", bufs=1, space="PSUM"))

    so = StreamOrder()
    pe, act, dve, pool = "PE", "ACT", "DVE", "POOL"

    # Pre-place one ACT table load of natural_log_exp_and_others (has Abs,
    # Ln, Exp, Copy): the bacc insertion pass then adds no further loads,
    # saving ~4us of serial table switching.
    set_id = [i for i, (n, _) in enumerate(get_activation_tables(nc.m.arch).items())
              if n == "natural_log_exp_and_others"][0]
    so.add(act, nc.scalar.add_instruction(mybir.InstLoadActFuncSet(
        name=nc.get_next_instruction_name(), ins=[], outs=[],
        act_func_set_id=set_id)))

    # DMA order: x pieces 0,1, mask (cast), x pieces 2,3 — x feeds the
    # long chain, the mask is only needed by the (later) matmuls.
    x_sb = big.tile([128, I], F32, tag="x")
    m_f16 = big.tile([128, I], FP16, tag="m")
    for g in range(N_GROUPS):
        nc.sync.dma_start(x_sb[:, g * GW:(g + 1) * GW], x_d[:, g * GW:(g + 1) * GW])
    nc.gpsimd.dma_start(m_f16[:], m_d[:])  # SWDGE casts f32->fp16 in flight

    # Identities after DMA issue (GPSIMD work overlaps the transfers).
    # fp16 identity first, f32 second: the warmup transpose (f32 ident, the
    # last GPSIMD product) lets the PE observe the whole GPSIMD tick range.
    identh = const.tile([128, 128], FP16)
    make_identity(nc, identh[:])
    ident = const.tile([128, 128], F32)
    make_identity(nc, ident[:])

    ax = big.tile([128, I], F32, tag="ax")        # |xT|
    lnax = big.tile([128, I], F32, tag="lnax")    # ln|x|, transposed layout
    maskT = big.tile([128, I], FP16, tag="maskT")  # mask shard transposed
    rhs = big.tile([128, N_CHUNKS * 256], FP16, tag="rhs")  # [hi|neg] per chunk
    rhs3 = rhs[:].rearrange("p (c n) -> p c n", n=256)

    pm = psm.tile([128, I], FP16)

    def x_group(g):
        gsl = slice(g * GW, (g + 1) * GW)
        cs = slice(GC * g, GC * g + GC)
        px = psx.tile([128, GW], F32, tag="px")
        for j in range(GC):
            so.add(pe, nc.tensor.transpose(px[:, ts(j, 128)], x_sb[:, ts(GC * g + j, 128)], ident[:]))
        pxv = px[:].rearrange("p (c n) -> p c n", n=128)
        lv = rhs3[:, cs, 0:128]
        # |x| pass alternates engines for load balance: even groups on ACT
        # (Abs is in the preloaded table set), odd groups on DVE (abs_max).
        if g % 2 == 0:
            so.add(act, nc.scalar.activation(ax[:, gsl], px[:], AF.Abs))
        else:
            so.add(dve, nc.vector.tensor_scalar(
                ax[:, gsl].bitcast(mybir.dt.int32), px[:].bitcast(mybir.dt.int32),
                0x7FFFFFFF, None, ALU.bitwise_and))  # |x| = clear sign bit
        # negative-factor indicators (exact in fp16)
        so.add(dve, nc.vector.tensor_scalar(rhs3[:, cs, 128:256], pxv, 0.0, None, ALU.is_lt))
        # Ln writes fp16 straight into the matmul moving operand.
        so.add(act, nc.scalar.activation(lv, ax[:, gsl], AF.Ln))

    # x groups 0-1 (first two DMA pieces), mask transposes + copies, 2-3.
    x_group(0)
    x_group(1)
    for j in range(N_CHUNKS):
        so.add(pe, nc.tensor.transpose(pm[:, ts(j, 128)], m_f16[:, ts(j, 128)], identh[:]))
    so.add(dve, nc.vector.tensor_copy(maskT[:, 0:I // 2], pm[:, 0:I // 2]))
    so.add(dve, nc.vector.tensor_copy(maskT[:, I // 2:I], pm[:, I // 2:I]))
    x_group(2)
    x_group(3)

    po = pso.tile([128, 256], F32, tag="po")
    for c in range(N_CHUNKS):
        so.add(pe, nc.tensor.matmul(
            po[:], maskT[:, ts(c, 128)], rhs3[:, c, :],
            start=(c == 0), stop=(c == N_CHUNKS - 1),
        ))

    # Epilogue on [128(o), 128(b)] tiles. po[:,0:128]=L, po[:,128:256]=N.
    # ACT is the first PSUM reader, DVE second (cross-engine PSUM reads
    # serialize in that order).
    t = smal.tile([128, B], F32, tag="t")
    so.add(act, nc.scalar.activation(t[:], po[:, 0:128], AF.Exp))
    pari = smal.tile([128, B], mybir.dt.int32, tag="pari")
    so.add(dve, nc.vector.tensor_copy(pari[:], po[:, 128:256]))  # exact count
    par = smal.tile([128, B], mybir.dt.int32, tag="par")
    so.add(dve, nc.vector.tensor_scalar(par[:], pari[:], 1, None, ALU.bitwise_and))
    sgn = smal.tile([128, B], F32, tag="sgn")
    so.add(dve, nc.vector.tensor_scalar(sgn[:], par[:], -2.0, 1.0, ALU.mult, ALU.add))
    # a = ln(1+t) needs no clip (t<=1 -> a<=ln2); only the 1-t side clips.
    a = smal.tile([128, B], F32, tag="a")
    so.add(act, nc.scalar.activation(a[:], t[:], AF.Ln, bias=1.0))
    t2 = smal.tile([128, B], F32, tag="t2")
    so.add(dve, nc.vector.tensor_scalar_min(t2[:], t[:], CLIP))
    bb = smal.tile([128, B], F32, tag="bb")
    so.add(act, nc.scalar.activation(bb[:], t2[:], AF.Ln, bias=1.0, scale=-1.0))
    u = smal.tile([128, B], F32, tag="u")
    so.add(dve, nc.vector.tensor_sub(u[:], a[:], bb[:]))
    ot = smal.tile([128, B], F32, tag="ot")
    so.add(dve, nc.vector.tensor_mul(ot[:], u[:], sgn[:]))
    nc.sync.dma_start(o_d[:], ot[:])


def build(loop_n: int = 0) -> bass.Bass:
    """Build the SPMD program. loop_n>0 wraps the body in a HW loop (timing)."""
    nc = bacc.Bacc("TRN2", target_bir_lowering=False, debug=False,
                   num_devices=N_CORES)
    x_d = nc.dram_tensor("x", [B, I], F32, kind="ExternalInput").ap()
    m_d = nc.dram_tensor("mask", [OS, I], F32, kind="ExternalInput").ap()
    o_d = nc.dram_tensor("outT", [OS, B], F32, kind="ExternalOutput").ap()
    with tile.TileContext(nc) as tc:
        with ExitStack() as ctx:
            if loop_n > 0:
                with tc.For_i(0, loop_n, 1):
                    build_body(ctx, tc, o_d, x_d, m_d)
            else:
                build_body(ctx, tc, o_d, x_d, m_d)
    nc.compile()
    return nc


_CACHE: dict = {}


def kernel(x: np.ndarray, mask: np.ndarray) -> np.ndarray:
    nc = _CACHE.get("nc")
    if nc is None:
        nc = _CACHE["nc"] = build()
    x = np.ascontiguousarray(np.asarray(x), dtype=np.float32)
    mask = np.ascontiguousarray(np.asarray(mask), dtype=np.float32)
    in_maps = [
        {"x": x, "mask": mask[c * OS:(c + 1) * OS]} for c in range(N_CORES)
    ]
    res = run_bass_kernel_spmd(nc, in_maps, list(range(N_CORES)))
    outT = np.concatenate(
        [res.results[c]["outT"] for c in range(N_CORES)], axis=0
    )  # [O, B]
    return np.ascontiguousarray(outT.T)


# revision 28
# speedup vs baseline: 1.7126x; 1.5849x over previous
"""Trainium2 Bass kernel for BeliefPropagationCV (LDPC check-node update).

Math: out[b,o] = 2*atanh(clip(prod_i (mask[o,i]*x[b,i] + 1-mask[o,i])))

The product over masked entries is computed in log-domain so it becomes two
matmuls over the Tanner graph mask:
    lnsq[b,i] = ln(x[b,i]^2) = 2*ln|x[b,i]|
    L2[b,o]   = sum_i mask[o,i]*lnsq[b,i]      (matmul)
    N[b,o]    = sum_i mask[o,i]*(x[b,i] < 0)   (matmul, negative-factor count)
    prod      = (-1)^N * exp(L2/2)
    out       = sign * (ln(1+t) - ln(1-t)),  t = min(exp(L2/2), 1-1e-7)

lnsq is split hi/lo into two bf16 matrices (hi = bf16(lnsq), lo = bf16(lnsq-hi))
so the matmuls run at full bf16 PE rate while retaining ~fp32 accuracy (the
mask is 0/1, exact in bf16; accumulation is fp32 in PSUM).

Sharding: output-dim (check-node rows of the mask) across 8 cores. Each core
gets the full x [128,2048] plus a [128,2048] row-shard of the mask, and
produces out.T shard [128(o),128(b)]. Host concatenates and transposes.
This minimizes HBM traffic (2MB/core) vs batch sharding (8.1MB/core).

Scheduling notes (walrus codegen allows ONE semaphore wait per engine
instruction): PSUM reads serialize cross-engine and pool-slot recycling waits
on all previous readers, so PSUM pools are sized to never recycle a slot
whose readers span two engines, and op emission order is chosen so each
instruction needs at most one new semaphore tick.
"""

import os
import sys
from contextlib import ExitStack

import numpy as np

for _p in ("/opt/trn_rl_repo", "/root/.axon_site/_ro/trn_rl_repo"):
    if os.path.isdir(_p) and _p not in sys.path:
        sys.path.append(_p)

import concourse.bacc as bacc
import concourse.bass as bass
import concourse.tile as tile
from concourse import mybir
from concourse.bass_utils import run_bass_kernel_spmd
from concourse.masks import make_identity
from concourse.hw_specs import get_activation_tables
from concourse.tile_rust import add_dep_helper


class StreamOrder:
    """Pins per-engine instruction order with nosync edges so the scheduler
    keeps emission order; semaphore waits then coalesce to <=1 per
    instruction (the walrus codegen limit)."""

    def __init__(self):
        self.last: dict = {}

    def add(self, key, binst):
        ins = getattr(binst, "ins", binst)
        prev = self.last.get(key)
        if prev is not None:
            add_dep_helper(ins, prev, sync=False, reason="stream-order")
        self.last[key] = ins
        return binst

N_CORES = 8
B = 128          # batch
O = 1024         # check nodes (mask rows)
I = 2048         # variable-node messages (mask cols)
OS = O // N_CORES  # mask rows per core

F32 = mybir.dt.float32
BF16 = mybir.dt.bfloat16
FP16 = mybir.dt.float16
AF = mybir.ActivationFunctionType
ALU = mybir.AluOpType
CLIP = float(np.float32(1.0) - np.float32(1e-7))

N_GROUPS = 4
GW = I // N_GROUPS   # 512 columns per x-group
GC = GW // 128       # 4 chunks per x-group
N_CHUNKS = I // 128  # 16 k-chunks of 128


def build_body(ctx: ExitStack, tc: "tile.TileContext", o_d, x_d, m_d):
    """Emit the per-core program. o_d: [OS,B] f32 out; x_d: [B,I]; m_d: [OS,I]."""
    nc = tc.nc
    ts = bass.ts

    const = ctx.enter_context(tc.tile_pool(name="const", bufs=1))
    big = ctx.enter_context(tc.tile_pool(name="big", bufs=1))
    smal = ctx.enter_context(tc.tile_pool(name="smal", bufs=1))
    # PSUM dep-tracking is per-TILE: one tile per group per operand, never
    # recycled. Budget (8 banks): px 4x[128,512]f32=4, pm [128,2048]fp16=2,
    # po=1, warmup=1.
    psx = ctx.enter_context(tc.tile_pool(name="psx", bufs=N_GROUPS, space="PSUM"))
    pso = ctx.enter_context(tc.tile_pool(name="pso", bufs=1, space="PSUM"))

    so = StreamOrder()
    pe, act, dve, pool = "PE", "ACT", "DVE", "POOL"

    # Pre-place one ACT table load of natural_log_exp_and_others (has Abs,
    # Ln, Exp, Copy): the bacc insertion pass then adds no further loads,
    # saving ~4us of serial table switching.
    set_id = [i for i, (n, _) in enumerate(get_activation_tables(nc.m.arch).items())
              if n == "natural_log_exp_and_others"][0]
    so.add(act, nc.scalar.add_instruction(mybir.InstLoadActFuncSet(
        name=nc.get_next_instruction_name(), ins=[], outs=[],
        act_func_set_id=set_id)))

    # x in 4 pieces (feeds the long chain piece by piece); maskT arrives
    # host-pre-transposed (static Tanner graph = weights prep) as fp16 in
    # chunk-column layout, ready to use as matmul weights.
    x_sb = big.tile([128, I], F32, tag="x")
    maskT = big.tile([128, I], FP16, tag="maskT")
    for g in range(N_GROUPS):
        nc.sync.dma_start(x_sb[:, g * GW:(g + 1) * GW], x_d[:, g * GW:(g + 1) * GW])
    nc.sync.dma_start(maskT[:], m_d[:])

    # Identities after DMA issue (GPSIMD work overlaps the transfers).
    # fp16 identity first, f32 second: the warmup transpose (f32 ident, the
    # last GPSIMD product) lets the PE observe the whole GPSIMD tick range.
    ident = const.tile([128, 128], F32)
    make_identity(nc, ident[:])

    ax = big.tile([128, I], F32, tag="ax")        # |xT|
    lnax = big.tile([128, I], F32, tag="lnax")    # ln|x|, transposed layout
    rhs = big.tile([128, N_CHUNKS * 256], FP16, tag="rhs")  # [hi|neg] per chunk
    rhs3 = rhs[:].rearrange("p (c n) -> p c n", n=256)


    def x_group(g):
        gsl = slice(g * GW, (g + 1) * GW)
        cs = slice(GC * g, GC * g + GC)
        px = psx.tile([128, GW], F32, tag="px")
        for j in range(GC):
            so.add(pe, nc.tensor.transpose(px[:, ts(j, 128)], x_sb[:, ts(GC * g + j, 128)], ident[:]))
        pxv = px[:].rearrange("p (c n) -> p c n", n=128)
        lv = rhs3[:, cs, 0:128]
        # |x| pass alternates engines for load balance: even groups on ACT
        # (Abs is in the preloaded table set), odd groups on DVE (abs_max).
        if g % 2 == 0:
            so.add(act, nc.scalar.activation(ax[:, gsl], px[:], AF.Abs))
        else:
            so.add(dve, nc.vector.tensor_scalar(
                ax[:, gsl].bitcast(mybir.dt.int32), px[:].bitcast(mybir.dt.int32),
                0x7FFFFFFF, None, ALU.bitwise_and))  # |x| = clear sign bit
        # negative-factor indicators (exact in fp16)
        so.add(dve, nc.vector.tensor_scalar(rhs3[:, cs, 128:256], pxv, 0.0, None, ALU.is_lt))
        # Ln writes fp16 straight into the matmul moving operand.
        so.add(act, nc.scalar.activation(lv, ax[:, gsl], AF.Ln))

    for g in range(N_GROUPS):
        x_group(g)

    po = pso.tile([128, 256], F32, tag="po")
    for c in range(N_CHUNKS):
        so.add(pe, nc.tensor.matmul(
            po[:], maskT[:, ts(c, 128)], rhs3[:, c, :],
            start=(c == 0), stop=(c == N_CHUNKS - 1),
        ))

    # Epilogue on [128(o), 128(b)] tiles. po[:,0:128]=L, po[:,128:256]=N.
    # ACT is the first PSUM reader, DVE second (cross-engine PSUM reads
    # serialize in that order).
    t = smal.tile([128, B], F32, tag="t")
    so.add(act, nc.scalar.activation(t[:], po[:, 0:128], AF.Exp))
    pari = smal.tile([128, B], mybir.dt.int32, tag="pari")
    so.add(dve, nc.vector.tensor_copy(pari[:], po[:, 128:256]))  # exact count
    par = smal.tile([128, B], mybir.dt.int32, tag="par")
    so.add(dve, nc.vector.tensor_scalar(par[:], pari[:], 1, None, ALU.bitwise_and))
    sgn = smal.tile([128, B], F32, tag="sgn")
    so.add(dve, nc.vector.tensor_scalar(sgn[:], par[:], -2.0, 1.0, ALU.mult, ALU.add))
    # a = ln(1+t) needs no clip (t<=1 -> a<=ln2); only the 1-t side clips.
    a = smal.tile([128, B], F32, tag="a")
    so.add(act, nc.scalar.activation(a[:], t[:], AF.Ln, bias=1.0))
    t2 = smal.tile([128, B], F32, tag="t2")
    so.add(dve, nc.vector.tensor_scalar_min(t2[:], t[:], CLIP))
    bb = smal.tile([128, B], F32, tag="bb")
    so.add(act, nc.scalar.activation(bb[:], t2[:], AF.Ln, bias=1.0, scale=-1.0))
    u = smal.tile([128, B], F32, tag="u")
    so.add(dve, nc.vector.tensor_sub(u[:], a[:], bb[:]))
    ot = smal.tile([128, B], F32, tag="ot")
    so.add(dve, nc.vector.tensor_mul(ot[:], u[:], sgn[:]))
    nc.sync.dma_start(o_d[:], ot[:])


def build(loop_n: int = 0) -> bass.Bass:
    """Build the SPMD program. loop_n>0 wraps the body in a HW loop (timing)."""
    nc = bacc.Bacc("TRN2", target_bir_lowering=False, debug=False,
                   num_devices=N_CORES)
    x_d = nc.dram_tensor("x", [B, I], F32, kind="ExternalInput").ap()
    m_d = nc.dram_tensor("mask", [128, I], FP16, kind="ExternalInput").ap()
    o_d = nc.dram_tensor("outT", [OS, B], F32, kind="ExternalOutput").ap()
    with tile.TileContext(nc) as tc:
        with ExitStack() as ctx:
            if loop_n > 0:
                with tc.For_i(0, loop_n, 1):
                    build_body(ctx, tc, o_d, x_d, m_d)
            else:
                build_body(ctx, tc, o_d, x_d, m_d)
    nc.compile()
    return nc


_CACHE: dict = {}


def kernel(x: np.ndarray, mask: np.ndarray) -> np.ndarray:
    nc = _CACHE.get("nc")
    if nc is None:
        nc = _CACHE["nc"] = build()
    x = np.ascontiguousarray(np.asarray(x), dtype=np.float32)
    mask = np.ascontiguousarray(np.asarray(mask), dtype=np.float32)
    in_maps = []
    for c in range(N_CORES):
        shard = mask[c * OS:(c + 1) * OS]  # [OS, I]
        # pre-transpose the static graph into fp16 chunk-column layout:
        # [:, k*128:(k+1)*128] = shard[:, k*128:(k+1)*128].T  (exact: 0/1)
        mT = np.concatenate(
            [shard[:, k * 128:(k + 1) * 128].T for k in range(I // 128)],
            axis=1).astype(np.float16)
        in_maps.append({"x": x, "mask": np.ascontiguousarray(mT)})
    res = run_bass_kernel_spmd(nc, in_maps, list(range(N_CORES)))
    outT = np.concatenate(
        [res.results[c]["outT"] for c in range(N_CORES)], axis=0
    )  # [O, B]
    return np.ascontiguousarray(outT.T)
